# revision 8
# baseline (speedup 1.0000x reference)
"""COVIDEENet Trainium2 kernel.

Sharding: 8 attention heads across 8 NeuronCores (tensor/head parallel).
Each core computes, for its head h:
    M   = WQ[h]^T @ WK[h]                                (e x e)
    A^T[e2,t] = sum_e1 M[e1,e2] E^T[e1,t]                (e x 1600)
    P^T[r]    = (E_r M E_r^T)^T                          (64 x 64), r = 0..24
    softmax over the industry axis fused with the business-structure
    weighted reduction -> BR^T (industries x districts) for its head.
BR tensors (tiny) are AllGathered across the 8 cores; BS (cosine over
heads) is computed replicated, CS (JSD) and OS (outbreak) are
data-parallel over the 25 target districts.  All final layernorms run on
device; the host only reshapes/transposes/gathers (index-driven layout).
"""

import numpy as np

R = 25          # regions / districts
C = 64          # companies (infected batch)
N = 64          # industries
E = 1024        # embedding dim
H = 8           # heads
NK = 27         # consumer categories
ECH = E // 128  # e chunks of 128
TL = R * N      # 1600
TC = 320        # t-chunk: 5 r-blocks, matmul free dim >= 256
NTC = TL // TC  # 5
RSLOT = 4       # stage-2 region slots per core
INV_SQRT_E = 1.0 / 32.0
LN_EPS = 1e-5
COS_EPS = 1e-15


def _regions_for_core(k):
    return [k + 8 * j if k + 8 * j < R else k for j in range(RSLOT)]


def _build_program(idx_t, idx_i):
    import concourse.mybir as mybir
    import concourse.tile as tile
    from concourse import bacc
    from contextlib import ExitStack

    dt = mybir.dt
    AX = mybir.AxisListType
    AL = mybir.AluOpType
    AF = mybir.ActivationFunctionType
    f32 = dt.float32
    f32r = dt.float32r

    nc = bacc.Bacc("TRN2", target_bir_lowering=False, debug=False, num_devices=8)

    def din(name, shape, dtype=f32):
        return nc.dram_tensor(name, list(shape), dtype, kind="ExternalInput").ap()

    def dout(name, shape, dtype=f32):
        return nc.dram_tensor(name, list(shape), dtype, kind="ExternalOutput").ap()

    ET_d = din("ET", [E, TL], f32r)            # normalized emb, transposed
    Wq_t_d = din("Wq_t", [E, E], f32r)         # natural (f, e) layout
    Wk_t_d = din("Wk_t", [E, E], f32r)
    Wq_i_d = din("Wq_i", [E, E], f32r)
    Wk_i_d = din("Wk_i", [E, E], f32r)
    bt_d = din("btf", [N, TL])                 # bt row-broadcast over industries
    bi_d = din("bif", [N, C * N])              # bi row-broadcast over industries
    ctT_d = din("ctT", [N, RSLOT * NK])        # ct[r].T per slot  [n, slot*27+k]
    ciT_d = din("ciT", [N, C * NK])            # ci transposed     [n, c*27+k]
    embT_os_d = din("embT_os", [E, RSLOT * N], f32r)  # raw emb^T slices for OS
    gobT_d = din("gobT", [E, C], f32r)         # raw emb rows gathered, transposed
    WosT_d = din("WosT", [E, E], f32r)         # W_os^T (f, e)
    bos_d = din("bos2d", [128, ECH])           # b_os reshaped [p, chunk]
    gb_d = din("gbT", [N, 6 * C])              # [BSg BSb CSg CSb OSg OSb]^T

    BS_d = dout("BS_out", [R, N, C])
    CS_d = dout("CS_out", [RSLOT, N, C])
    OS_d = dout("OS_out", [RSLOT, N, C])

    idx_t = [int(v) for v in idx_t]
    idx_i = [int(v) for v in idx_i]

    with tile.TileContext(nc) as tc, ExitStack() as ctx:
        pconst = ctx.enter_context(tc.tile_pool(name="pconst", bufs=1))
        pw = ctx.enter_context(tc.tile_pool(name="pw", bufs=1))
        pwq = ctx.enter_context(tc.tile_pool(name="pwq", bufs=16))
        pm = ctx.enter_context(tc.tile_pool(name="pm", bufs=1))
        pet = ctx.enter_context(tc.tile_pool(name="pet", bufs=2))
        pa = ctx.enter_context(tc.tile_pool(name="pa", bufs=1))
        pbbc = ctx.enter_context(tc.tile_pool(name="pbbc", bufs=1))
        psm = ctx.enter_context(tc.tile_pool(name="psm", bufs=1))
        pscr = ctx.enter_context(tc.tile_pool(name="pscr", bufs=3))
        pcs = ctx.enter_context(tc.tile_pool(name="pcs", bufs=2))
        pfin = ctx.enter_context(tc.tile_pool(name="pfin", bufs=1))
        pbigp = ctx.enter_context(tc.tile_pool(name="pbigp", bufs=3, space="PSUM"))
        psmp = ctx.enter_context(tc.tile_pool(name="psmp", bufs=2, space="PSUM"))
        pdram = ctx.enter_context(tc.tile_pool(name="pdram", bufs=1, space="DRAM"))

        # ---------------- constants / small loads ----------------
        gb_sb = pconst.tile([N, 6 * C], f32)
        nc.sync.dma_start(gb_sb[:], gb_d[:])
        onesS = pconst.tile([C, 1], f32)
        nc.vector.memset(onesS[:], 1.0 / 4096.0)
        onesR = pconst.tile([1, C], f32)
        nc.vector.memset(onesR[:], 1.0)
        bos_sb = pconst.tile([128, ECH], f32)
        nc.sync.dma_start(bos_sb[:], bos_d[:])
        gobT_sb = pconst.tile([128, ECH * C], f32r)
        for k in range(ECH):
            nc.sync.dma_start(
                gobT_sb[:, k * C:(k + 1) * C],
                gobT_d[k * 128:(k + 1) * 128, :],
            )

        # ---------------- CS: JSD customer-structure similarity ----------------
        # Emitted as deferred chunks interleaved into the attention pipeline so
        # the DVE/ACT queues stay behind the matmul PSUM drains.
        # layout: partitions = n (64), free = (c, 27) / (slot, 27), c-major.
        CH = 16            # companies per quarter-pass
        CW = CH * NK       # 432
        NHF = C // CH      # 4
        cs_state = {}
        CSpre = pfin.tile([N, RSLOT * C], f32, tag="cspre")

        def cs_chunk_prelude():
            ctT_sb = pcs.tile([N, RSLOT * NK], f32, tag="ct")
            nc.sync.dma_start(ctT_sb[:], ctT_d[:])
            LT = pcs.tile([N, RSLOT * NK], f32, tag="lt", bufs=1)
            e1 = pcs.tile([N, RSLOT * NK], f32, tag="ct")
            nc.scalar.activation(e1[:], ctT_sb[:], AF.Exp)
            s1 = pcs.tile([N, RSLOT], f32, tag="s", bufs=4)
            nc.vector.tensor_reduce(s1[:], e1.rearrange("p (s k) -> p s k", k=NK),
                                    axis=AX.X, op=AL.add)
            l1 = pcs.tile([N, RSLOT], f32, tag="s", bufs=4)
            nc.scalar.activation(l1[:], s1[:], AF.Ln)
            nc.vector.tensor_tensor(
                LT.rearrange("p (s k) -> p s k", k=NK),
                ctT_sb.rearrange("p (s k) -> p s k", k=NK),
                l1[:, :, None].broadcast_to([N, RSLOT, NK]),
                op=AL.subtract,
            )
            cs_state["LT"] = LT

        def cs_chunk_li(hf):
            def emit():
                cih = pcs.tile([N, CW], f32, tag="ci", bufs=1, name=f"cih_{hf}")
                nc.sync.dma_start(cih[:], ciT_d[:, hf * CW:(hf + 1) * CW])
                eh = pcs.tile([N, CW], f32, tag="x", bufs=1, name=f"eh_{hf}")
                nc.scalar.activation(eh[:], cih[:], AF.Exp)
                sh = pcs.tile([N, CH], f32, tag="s", bufs=4, name=f"sh_{hf}")
                nc.vector.tensor_reduce(sh[:], eh.rearrange("p (c k) -> p c k", k=NK),
                                        axis=AX.X, op=AL.add)
                lh = pcs.tile([N, CH], f32, tag="s", bufs=4, name=f"lh_{hf}")
                nc.scalar.activation(lh[:], sh[:], AF.Ln)
                li = pcs.tile([N, CW], f32, tag=f"li{hf}", bufs=1, name=f"li_{hf}")
                nc.vector.tensor_tensor(
                    li.rearrange("p (c k) -> p c k", k=NK),
                    cih.rearrange("p (c k) -> p c k", k=NK),
                    lh[:, :, None].broadcast_to([N, CH, NK]),
                    op=AL.subtract,
                )
                cs_state[f"li{hf}"] = li
            return emit

        def cs_chunk_slot(s, hf):
            def emit():
                LT = cs_state["LT"]
                li = cs_state[f"li{hf}"]
                if hf == 0:
                    cs_state[f"KT{s}"] = pcs.tile([N, C], f32, tag="kt", bufs=2,
                                                  name=f"KT_{s}")
                    cs_state[f"KI{s}"] = pcs.tile([N, C], f32, tag="ki", bufs=2,
                                                  name=f"KI_{s}")
                KT = cs_state[f"KT{s}"]
                KI = cs_state[f"KI{s}"]
                lts = LT[:, s * NK:(s + 1) * NK]
                X = pcs.tile([N, CW], f32, tag="x", bufs=1, name=f"X_{s}_{hf}")
                nc.vector.tensor_tensor(
                    X.rearrange("p (c k) -> p c k", k=NK),
                    li.rearrange("p (c k) -> p c k", k=NK),
                    lts[:, None, :].broadcast_to([N, CH, NK]),
                    op=AL.add,
                )
                E2 = pcs.tile([N, CW], f32, tag="e2", bufs=1, name=f"E2_{s}_{hf}")
                nc.scalar.activation(E2[:], X[:], AF.Exp, scale=0.5)
                s2 = pcs.tile([N, CH], f32, tag="s", bufs=4, name=f"s2_{s}_{hf}")
                nc.vector.tensor_reduce(s2[:], E2.rearrange("p (c k) -> p c k", k=NK),
                                        axis=AX.X, op=AL.add)
                L2 = pcs.tile([N, CH], f32, tag="s", bufs=4, name=f"L2_{s}_{hf}")
                nc.scalar.activation(L2[:], s2[:], AF.Ln)
                Mh = pcs.tile([N, CW], f32, tag="mh", bufs=1, name=f"Mh_{s}_{hf}")
                nc.vector.scalar_tensor_tensor(
                    Mh.rearrange("p (c k) -> p c k", k=NK),
                    X.rearrange("p (c k) -> p c k", k=NK),
                    0.5,
                    L2[:, :, None].broadcast_to([N, CH, NK]),
                    op0=AL.mult, op1=AL.subtract,
                )
                EM = pcs.tile([N, CW], f32, tag="em", bufs=1, name=f"EM_{s}_{hf}")
                nc.scalar.activation(EM[:], Mh[:], AF.Exp)
                t1 = pcs.tile([N, CW], f32, tag="t1", bufs=1, name=f"t1_{s}_{hf}")
                nc.vector.tensor_tensor(
                    t1.rearrange("p (c k) -> p c k", k=NK),
                    Mh.rearrange("p (c k) -> p c k", k=NK),
                    lts[:, None, :].broadcast_to([N, CH, NK]),
                    op=AL.subtract,
                )
                nc.vector.tensor_tensor(t1[:], EM[:], t1[:], op=AL.mult)
                nc.vector.tensor_reduce(KT[:, hf * CH:(hf + 1) * CH],
                                        t1.rearrange("p (c k) -> p c k", k=NK),
                                        axis=AX.X, op=AL.add)
                nc.vector.tensor_tensor(t1[:], Mh[:], li[:], op=AL.subtract)
                nc.vector.tensor_tensor(t1[:], EM[:], t1[:], op=AL.mult)
                nc.vector.tensor_reduce(KI[:, hf * CH:(hf + 1) * CH],
                                        t1.rearrange("p (c k) -> p c k", k=NK),
                                        axis=AX.X, op=AL.add)
                if hf == NHF - 1:
                    nc.vector.tensor_tensor(KT[:], KT[:], KI[:], op=AL.add)
                    nc.vector.tensor_scalar_mul(CSpre[:, s * C:(s + 1) * C], KT[:],
                                                -1.0 / (2.0 * NK))
            return emit

        cs_chunks = [cs_chunk_prelude] + [cs_chunk_li(q) for q in range(NHF)]
        for s in range(RSLOT):
            for q in range(NHF):
                cs_chunks.append(cs_chunk_slot(s, q))
        cs_chunks.reverse()   # pop() from the front

        def filler():
            if cs_chunks:
                cs_chunks.pop()()

        # ---------------- attention pipelines (the compute core) ----------------
        def pipeline(tag, Wq_d, Wk_d, bbc_d, nd, idx):
            """Returns BR^T tile (64 industry partitions, nd district columns)."""
            bbc = pbbc.tile([N, C * N], f32, tag="bbc", name=f"bbc_{tag}")
            nc.sync.dma_start(bbc[:, 0:nd * N], bbc_d[:])
            WK = pw.tile([128, ECH * E], f32r, tag="wk", name=f"wk_{tag}")
            for k in range(ECH):
                nc.sync.dma_start(WK[:, k * E:(k + 1) * E],
                                  Wk_d[k * 128:(k + 1) * 128, :])
            M_sb = pm.tile([128, ECH * E], f32r, tag="m", name=f"m_{tag}")
            for m in range(ECH):
                wqs = []
                for k in range(ECH):
                    wq = pwq.tile([128, 128], f32r, tag="wq", name=f"wq_{tag}_{m}_{k}")
                    nc.sync.dma_start(
                        wq[:],
                        Wq_d[k * 128:(k + 1) * 128, m * 128:(m + 1) * 128])
                    wqs.append(wq)
                for n2 in range(2):
                    ps = pbigp.tile([128, 512], f32, tag="mm", name=f"psm_{tag}_{m}_{n2}")
                    for k in range(ECH):
                        nc.tensor.matmul(ps[:], wqs[k][:],
                                         WK[:, k * E + n2 * 512:k * E + (n2 + 1) * 512],
                                         start=(k == 0), stop=(k == ECH - 1))
                    nc.vector.tensor_copy(
                        M_sb[:, m * E + n2 * 512:m * E + (n2 + 1) * 512], ps[:])
                filler()

            expS = psm.tile([N, TL], f32, tag="exps", name=f"expS_{tag}")
            DEN = psm.tile([N, R], f32, tag=f"den_{tag}", name=f"DEN_{tag}")
            for tcn in range(NTC):
                ETt = pet.tile([128, ECH * TC], f32r, tag="et", name=f"et_{tag}_{tcn}")
                for k in range(ECH):
                    nc.sync.dma_start(
                        ETt[:, k * TC:(k + 1) * TC],
                        ET_d[k * 128:(k + 1) * 128,
                             tcn * TC:(tcn + 1) * TC])
                At = pa.tile([128, ECH * TC], f32r, tag="a", name=f"a_{tag}_{tcn}")
                for m in range(ECH):
                    ps = pbigp.tile([128, TC], f32, tag="mm",
                                    name=f"psa_{tag}_{tcn}_{m}")
                    for k in range(ECH):
                        nc.tensor.matmul(ps[:],
                                         M_sb[:, k * E + m * 128:k * E + (m + 1) * 128],
                                         ETt[:, k * TC:(k + 1) * TC],
                                         start=(k == 0), stop=(k == ECH - 1))
                    nc.vector.tensor_copy(
                        At[:, m * TC:(m + 1) * TC], ps[:])
                filler()
                for rr in range(TC // N):
                    r = tcn * (TC // N) + rr
                    pp = psmp.tile([N, N], f32, tag="pp", name=f"pp_{tag}_{r}")
                    for k in range(ECH):
                        nc.tensor.matmul(
                            pp[:],
                            ETt[:, k * TC + rr * N:k * TC + (rr + 1) * N],
                            At[:, k * TC + rr * N:k * TC + (rr + 1) * N],
                            start=(k == 0), stop=(k == ECH - 1))
                    nc.scalar.activation(expS[:, r * N:(r + 1) * N], pp[:], AF.Exp,
                                         scale=INV_SQRT_E,
                                         accum_out=DEN[:, r:r + 1])

            NUM = psm.tile([N, nd], f32, tag=f"num_{tag}", name=f"NUM_{tag}")
            BR = psm.tile([N, nd], f32, tag=f"br_{tag}", name=f"BR_{tag}")
            for d in range(nd):
                r = idx[d]
                scr = pscr.tile([N, N], f32, tag="scr", name=f"scr_{tag}_{d}")
                nc.vector.tensor_tensor(scr[:], expS[:, r * N:(r + 1) * N],
                                        bbc[:, d * N:(d + 1) * N], op=AL.mult)
                nc.vector.tensor_reduce(NUM[:, d:d + 1], scr[:],
                                        axis=AX.X, op=AL.add)
            RDEN = psm.tile([N, R], f32, tag="rden", name=f"RDEN_{tag}")
            nc.vector.reciprocal(RDEN[:], DEN[:])
            for d in range(nd):
                r = idx[d]
                nc.vector.tensor_tensor(BR[:, d:d + 1], NUM[:, d:d + 1],
                                        RDEN[:, r:r + 1], op=AL.mult)
            return BR

        BRt = pipeline("t", Wq_t_d, Wk_t_d, bt_d, R, idx_t)
        BRi = pipeline("i", Wq_i_d, Wk_i_d, bi_d, C, idx_i)
        while cs_chunks:
            filler()

        # ---------------- OS: outbreak-business similarity ----------------
        # ob^T[e, c] = sum_f W_os^T[f, e] * gob^T[f, c]  (+ b_os)
        obT = pfin.tile([128, ECH * C], f32r, tag="obt")
        for ec in range(ECH):
            wos = []
            for k in range(ECH):
                w = pwq.tile([128, 128], f32r, tag="wq", name=f"wos_{ec}_{k}")
                nc.sync.dma_start(
                    w[:],
                    WosT_d[k * 128:(k + 1) * 128, ec * 128:(ec + 1) * 128])
                wos.append(w)
            ps = psmp.tile([128, C], f32, tag="pso", name=f"pso_{ec}")
            for k in range(ECH):
                nc.tensor.matmul(ps[:], wos[k][:], gobT_sb[:, k * C:(k + 1) * C],
                                 start=(k == 0), stop=(k == ECH - 1))
            nc.scalar.activation(obT[:, ec * C:(ec + 1) * C], ps[:],
                                 AF.Identity, bias=bos_sb[:, ec:ec + 1])

        OSpre = pfin.tile([N, RSLOT * C], f32, tag="ospre")
        for s in range(RSLOT):
            embs = []
            for k in range(ECH):
                w = pwq.tile([128, N], f32r, tag="wq", name=f"wemb_{s}_{k}")
                nc.sync.dma_start(
                    w[:],
                    embT_os_d[k * 128:(k + 1) * 128, s * N:(s + 1) * N])
                embs.append(w)
            ps = psmp.tile([N, C], f32, tag="pso", name=f"psos_{s}")
            for k in range(ECH):
                nc.tensor.matmul(
                    ps[:], embs[k][:], obT[:, k * C:(k + 1) * C],
                    start=(k == 0), stop=(k == ECH - 1))
            nc.vector.tensor_copy(OSpre[:, s * C:(s + 1) * C], ps[:])

        # ---------------- layernorm helpers ----------------
        def stats_cols(pre, nslots, stat, base):
            """stat (f32r) gets [sums | sumsqs] over each slot's (n, c) block."""
            nc.vector.tensor_reduce(stat[:, base:base + nslots],
                                    pre.rearrange("p (s c) -> p s c", c=C),
                                    axis=AX.X, op=AL.add)
            sq = pscr.tile([N, nslots * C], f32, tag="sq", bufs=1,
                           name=f"sq_{base}_{nslots}")
            nc.scalar.activation(sq[:], pre[:], AF.Square)
            nc.vector.tensor_reduce(stat[:, base + nslots:base + 2 * nslots],
                                    sq.rearrange("p (s c) -> p s c", c=C),
                                    axis=AX.X, op=AL.add)

        def ln_finalize(statb, nslots, base):
            mean = statb[:, base:base + nslots]
            ex2 = statb[:, base + nslots:base + 2 * nslots]
            m2 = pscr.tile([N, nslots], f32, tag="lnt", bufs=4, name=f"m2_{base}_{nslots}")
            nc.scalar.activation(m2[:], mean, AF.Square)
            var = pscr.tile([N, nslots], f32, tag="lnt", bufs=4,
                            name=f"var_{base}_{nslots}")
            nc.vector.tensor_tensor(var[:], ex2, m2[:], op=AL.subtract)
            nc.vector.tensor_scalar_add(var[:], var[:], LN_EPS)
            sd = pscr.tile([N, nslots], f32, tag="lnt", bufs=4, name=f"sd_{base}_{nslots}")
            nc.scalar.activation(sd[:], var[:], AF.Sqrt)
            rstd = pscr.tile([N, nslots], f32, tag="lnt", bufs=4,
                             name=f"rstd_{base}_{nslots}")
            nc.vector.reciprocal(rstd[:], sd[:])
            return mean, rstd

        def ln_apply_store(pre, s, mean, rstd, gsl, bsl, out_d, nm):
            t3 = pscr.tile([N, C], f32, tag="lnap", bufs=3, name=f"ln_{nm}_{s}")
            nc.vector.tensor_tensor(t3[:], pre[:, s * C:(s + 1) * C],
                                    mean[:, s:s + 1].broadcast_to([N, C]),
                                    op=AL.subtract)
            nc.vector.tensor_tensor(t3[:], t3[:],
                                    rstd[:, s:s + 1].broadcast_to([N, C]),
                                    op=AL.mult)
            nc.vector.tensor_tensor(t3[:], t3[:], gb_sb[:, gsl * C:(gsl + 1) * C],
                                    op=AL.mult)
            nc.vector.tensor_tensor(t3[:], t3[:], gb_sb[:, bsl * C:(bsl + 1) * C],
                                    op=AL.add)
            nc.sync.dma_start(out_d[s], t3[:])

        # CS/OS layernorm (ones-matmul partition sum -> broadcast)
        STATCO = pfin.tile([N, 4 * RSLOT], f32, tag="statco")
        stats_cols(CSpre, RSLOT, STATCO, 0)
        stats_cols(OSpre, RSLOT, STATCO, 2 * RSLOT)
        psst = psmp.tile([1, 4 * RSLOT], f32, tag="pst", bufs=1, name="psst_co")
        nc.tensor.matmul(psst[:], onesS[:, :1], STATCO[:], start=True, stop=True)
        rowCO = pfin.tile([1, 4 * RSLOT], f32, tag="rowco")
        nc.vector.tensor_copy(rowCO[:], psst[:])
        STATBCO = pfin.tile([N, 4 * RSLOT], f32, tag="statbco")
        psb1 = psmp.tile([N, 4 * RSLOT], f32, tag="pst", bufs=1, name="psb_co")
        nc.tensor.matmul(psb1[:], onesR[:1, :N], rowCO[:1, :], start=True, stop=True)
        nc.vector.tensor_copy(STATBCO[:], psb1[:])
        mean_cs, rstd_cs = ln_finalize(STATBCO, RSLOT, 0)
        mean_os, rstd_os = ln_finalize(STATBCO, RSLOT, 2 * RSLOT)
        for s in range(RSLOT):
            ln_apply_store(CSpre, s, mean_cs, rstd_cs, 2, 3, CS_d, "cs")
            ln_apply_store(OSpre, s, mean_os, rstd_os, 4, 5, OS_d, "os")

        # ---------------- AllGather BR across heads ----------------
        AGW = N * R + N * C  # 5696 floats per rank
        cin = pdram.tile([AGW], f32)
        nc.sync.dma_start(cin[0:N * R].rearrange("(a b) -> a b", a=N), BRt[:])
        nc.sync.dma_start(cin[N * R:AGW].rearrange("(a b) -> a b", a=N), BRi[:])
        agout = pdram.tile([H, AGW], f32, addr_space="Shared")
        nc.gpsimd.collective_compute(
            "AllGather", mybir.AluOpType.bypass,
            replica_groups=[list(range(H))],
            ins=[cin.opt()], outs=[agout.opt()],
        )

        # ---------------- BS: cosine similarity over heads ----------------
        TRG = pfin.tile([N, H * R], f32, tag="trg")   # [j, h*25+r]
        INF = pfin.tile([N, H * C], f32, tag="inf")   # [j, h*64+c]
        for h in range(H):
            nc.sync.dma_start(TRG[:, h * R:(h + 1) * R],
                              agout[h, 0:N * R].rearrange("(a b) -> a b", a=N))
            nc.sync.dma_start(INF[:, h * C:(h + 1) * C],
                              agout[h, N * R:AGW].rearrange("(a b) -> a b", a=N))

        def inv_norms(src, cols, nm):
            sq = pscr.tile([N, H * cols], f32, tag="sq", bufs=1, name=f"nsq_{nm}")
            nc.scalar.activation(sq[:], src[:], AF.Square)
            nsq = pfin.tile([N, cols], f32, tag=f"nrm_{nm}", name=f"nrm_{nm}")
            nc.vector.tensor_reduce(nsq[:], sq.rearrange("p (h r) -> p r h", h=H),
                                    axis=AX.X, op=AL.add)
            nc.scalar.activation(nsq[:], nsq[:], AF.Sqrt)
            nc.vector.tensor_scalar_max(nsq[:], nsq[:], COS_EPS)
            nc.vector.reciprocal(nsq[:], nsq[:])
            return nsq

        RNA = inv_norms(TRG, R, "a")   # (64, 25)
        RNB = inv_norms(INF, C, "b")   # (64, 64)

        BSpre = pfin.tile([N, R * C], f32, tag="bspre")
        trg_v = TRG.rearrange("p (h r) -> p r h", h=H)
        inf_v = INF.rearrange("p (h c) -> p c h", h=H)
        for r in range(R):
            tmp = pscr.tile([N, C * H], f32, tag="bst", bufs=1, name=f"bst_{r}")
            nc.vector.tensor_tensor(
                tmp.rearrange("p (c h) -> p c h", h=H),
                inf_v,
                trg_v[:, r:r + 1, :].broadcast_to([N, C, H]),
                op=AL.mult)
            dot = pscr.tile([N, C], f32, tag="bsd", bufs=2, name=f"bsdot_{r}")
            nc.vector.tensor_reduce(dot[:], tmp.rearrange("p (c h) -> p c h", h=H),
                                    axis=AX.X, op=AL.add)
            nc.vector.tensor_tensor(dot[:], dot[:], RNB[:], op=AL.mult)
            nc.vector.tensor_tensor(BSpre[:, r * C:(r + 1) * C], dot[:],
                                    RNA[:, r:r + 1].broadcast_to([N, C]),
                                    op=AL.mult)

        STATBS = pfin.tile([N, 2 * R], f32, tag="statbs")
        stats_cols(BSpre, R, STATBS, 0)
        psbs = psmp.tile([1, 2 * R], f32, tag="pst", bufs=1, name="psst_bs")
        nc.tensor.matmul(psbs[:], onesS[:, :1], STATBS[:], start=True, stop=True)
        rowBS = pfin.tile([1, 2 * R], f32, tag="rowbs")
        nc.vector.tensor_copy(rowBS[:], psbs[:])
        STATBBS = pfin.tile([N, 2 * R], f32, tag="statbbs")
        psb2 = psmp.tile([N, 2 * R], f32, tag="pst", bufs=1, name="psb_bs")
        nc.tensor.matmul(psb2[:], onesR[:1, :N], rowBS[:1, :], start=True, stop=True)
        nc.vector.tensor_copy(STATBBS[:], psb2[:])
        mean_bs, rstd_bs = ln_finalize(STATBBS, R, 0)
        for r in range(R):
            ln_apply_store(BSpre, r, mean_bs, rstd_bs, 0, 1, BS_d, "bs")

    nc.compile()
    return nc


def kernel(**inputs):
    from concourse import bass_utils

    f32 = np.float32
    bst = np.asarray(inputs["business_structure_target"], f32)
    bsi = np.asarray(inputs["business_structure_infected"], f32)
    cst = np.asarray(inputs["customer_structure_target"], f32)
    csi = np.asarray(inputs["customer_structure_infected"], f32)
    idx_t = np.asarray(inputs["index_target_idx"]).astype(np.int64)[:R, 0]
    idx_i = np.asarray(inputs["index_infected_idx"]).astype(np.int64)[0]
    cov = np.asarray(inputs["covid_outbreak_business"]).astype(np.int64)[0]
    emb = np.asarray(inputs["emb_weight"], f32)
    emb_g = np.asarray(inputs["emb_ln_g"], f32)
    emb_b = np.asarray(inputs["emb_ln_b"], f32)
    WQ_t = np.asarray(inputs["WQ_t"], f32)
    WK_t = np.asarray(inputs["WK_t"], f32)
    WQ_i = np.asarray(inputs["WQ_i"], f32)
    WK_i = np.asarray(inputs["WK_i"], f32)
    W_os = np.asarray(inputs["W_os"], f32)
    b_os = np.asarray(inputs["b_os"], f32)
    gbs = [np.asarray(inputs[k], f32) for k in
           ("BS_g", "BS_b", "CS_g", "CS_b", "OS_g", "OS_b")]

    # host prep: means over the 4-sample axis, slicing, emb layernorm, layouts
    bt = bst.mean(-1)[:R, 0]                    # (25, 64)
    bi = bsi.mean(-1)[0]                        # (64, 64)
    ct = cst.mean(-1)[:R, 0]                    # (25, 64, 27)
    ci = csi.mean(-1)[0]                        # (64, 64, 27)

    em64 = emb.astype(np.float64)
    mu = em64.mean(1, keepdims=True)
    va = ((em64 - mu) ** 2).mean(1, keepdims=True)
    En = ((em64 - mu) / np.sqrt(va + 1e-16) * emb_g + emb_b).astype(f32)
    ET = np.ascontiguousarray(En.T)             # (1024, 1600)

    inf_emb_idx = (idx_i * N + cov).astype(np.int64)
    gobT = np.ascontiguousarray(emb[inf_emb_idx].T)         # (1024, 64)
    WosT = np.ascontiguousarray(W_os.T)                     # (1024, 1024)
    bos2d = np.ascontiguousarray(b_os.reshape(ECH, 128).T)  # (128, 8)
    gbT = np.concatenate([np.ascontiguousarray(g.T) for g in gbs], axis=1)
    ciT = np.ascontiguousarray(ci.transpose(1, 0, 2).reshape(N, C * NK))

    btbF = np.ascontiguousarray(np.tile(bt.reshape(1, -1), (N, 1)))
    bibF = np.ascontiguousarray(np.tile(bi.reshape(1, -1), (N, 1)))

    nc = _build_program(idx_t, idx_i)

    in_maps = []
    for k in range(8):
        regions = _regions_for_core(k)
        ctT = np.ascontiguousarray(
            ct[regions].transpose(1, 0, 2).reshape(N, RSLOT * NK))
        embT_os = np.ascontiguousarray(
            np.concatenate([emb[r * N:(r + 1) * N] for r in regions], 0).T)
        in_maps.append({
            "ET": ET,
            "Wq_t": np.ascontiguousarray(WQ_t[k]),
            "Wk_t": np.ascontiguousarray(WK_t[k]),
            "Wq_i": np.ascontiguousarray(WQ_i[k]),
            "Wk_i": np.ascontiguousarray(WK_i[k]),
            "btf": btbF,
            "bif": bibF,
            "ctT": ctT,
            "ciT": ciT,
            "embT_os": embT_os,
            "gobT": gobT,
            "WosT": WosT,
            "bos2d": bos2d,
            "gbT": gbT,
        })

    res = bass_utils.run_bass_kernel_spmd(nc, in_maps, core_ids=list(range(8)))

    BS = np.ascontiguousarray(res.results[0]["BS_out"].transpose(0, 2, 1))
    CS = np.empty((R, C, N), f32)
    OS = np.empty((R, C, N), f32)
    for r in range(R):
        k, j = r % 8, r // 8
        CS[r] = res.results[k]["CS_out"][j].T
        OS[r] = res.results[k]["OS_out"][j].T
    return (BS, CS, OS)
